# revision 1
# baseline (speedup 1.0000x reference)
"""Causal cross-attention Trainium2 kernel.

Sharding: 8 cores = 2 (batch) x 4 (head-groups of 4 heads, tensor parallel).
Each core computes its batch's attention for its 4 heads plus the partial
(row-parallel) output projection; the host sums head-group partials + bo.

Device layout is fully transposed (Q^T/K^T = [d, seq], scores = S^T[k, q],
output Y^T = [D, SQ]) so that:
  - the softmax denominator comes free as a fused ones-column in the PV matmul
  - Q/K biases are per-partition adds
  - the out-projection streams OT as the moving operand (few weight loads)
No max-subtraction in softmax: scores are ~N(0,1) (scale 1/sqrt(64) folded
into Wq), so raw exp is safe in fp32.

Attention processes head pairs concurrently: the two K=64 score matmuls are
row-packed into disjoint PE row groups (partitions 0-63 / 64-127), their
outputs share one 2-bank PSUM tile, and a single strided ACT exp covers both.
"""

import sys

if "/opt/trn_rl_repo" not in sys.path:
    sys.path.insert(0, "/opt/trn_rl_repo")

import numpy as np

import concourse.mybir as mybir
import concourse.tile as tile
from concourse import bacc
from concourse.bass_utils import run_bass_kernel_spmd

# problem shapes (hardcoded)
B = 2
SQ = 2048
SKV = 2048
D = 1024
H = 16
DH = 64
N_CORES = 8
HG = 4  # head groups
H_PER = H // HG  # 4 heads per core
DG = H_PER * DH  # 256 dims per core

F16 = mybir.dt.float16
F32 = mybir.dt.float32

QB = 512  # q block (free dim per matmul)
KT = 128  # kv tile (partition dim)
NQB = SQ // QB  # 4
NKT = SKV // KT  # 16
NCT = D // 128  # 8 contraction tiles for projections
NDT = DG // 128  # 2 partition tiles for the 256 head dims

_CACHE = {}


def _build():
    nc = bacc.Bacc("TRN2", target_bir_lowering=False, debug=False)

    qT_ext = nc.dram_tensor("qT", [D, SQ], F16, kind="ExternalInput")
    kvT_ext = nc.dram_tensor("kvT", [D, SKV], F16, kind="ExternalInput")
    wqT_ext = nc.dram_tensor("wqT", [D, DG], F16, kind="ExternalInput")
    wkT_ext = nc.dram_tensor("wkT", [D, DG], F16, kind="ExternalInput")
    wvT_ext = nc.dram_tensor("wvT", [D, DG], F16, kind="ExternalInput")
    woT_ext = nc.dram_tensor("woT", [DG, D], F16, kind="ExternalInput")
    bqk_ext = nc.dram_tensor("bqk", [128, 2 * NDT], F32, kind="ExternalInput")
    bv_ext = nc.dram_tensor("bv", [1, DG], F16, kind="ExternalInput")
    mask_ext = nc.dram_tensor("mask", [128, 128], F16, kind="ExternalInput")
    yT_ext = nc.dram_tensor("yT", [D, SQ], F16, kind="ExternalOutput")

    with tile.TileContext(nc) as tc:
        with (
            tc.tile_pool(name="res", bufs=1) as res,
            tc.tile_pool(name="pp", bufs=6) as pp,
            tc.tile_pool(name="yp", bufs=6) as yp,
            tc.tile_pool(name="rp", bufs=3) as rp,
            tc.tile_pool(name="op", bufs=5) as op,
        ):
            # ---- weights first (small), then inputs tiled per kt ----------
            wq_s = res.tile([128, NCT, DG], F16)
            nc.sync.dma_start(
                out=wq_s, in_=wqT_ext.rearrange("(kt p) d -> p kt d", p=128)
            )
            wk_s = res.tile([128, NCT, DG], F16)
            nc.sync.dma_start(
                out=wk_s, in_=wkT_ext.rearrange("(kt p) d -> p kt d", p=128)
            )
            qT_kt = []
            kvT_kt = []
            qT_view = qT_ext.rearrange("(kt p) q -> kt p q", p=128)
            kvT_view = kvT_ext.rearrange("(kt p) q -> kt p q", p=128)
            for kt in range(NCT):
                qt_t = res.tile([128, SQ], F16, tag=f"qT{kt}", name=f"qT{kt}")
                nc.sync.dma_start(out=qt_t, in_=qT_view[kt])
                qT_kt.append(qt_t)
                kvt_t = res.tile(
                    [128, SKV], F16, tag=f"kvT{kt}", name=f"kvT{kt}"
                )
                nc.sync.dma_start(out=kvt_t, in_=kvT_view[kt])
                kvT_kt.append(kvt_t)
            wv_s = res.tile([128, NCT, DG], F16)
            nc.sync.dma_start(
                out=wv_s, in_=wvT_ext.rearrange("(kt p) d -> p kt d", p=128)
            )
            wo_s = res.tile([128, NDT, D], F16)
            nc.sync.dma_start(
                out=wo_s, in_=woT_ext.rearrange("(kt p) m -> p kt m", p=128)
            )
            bqk_s = res.tile([128, 2 * NDT], F32)
            nc.sync.dma_start(out=bqk_s, in_=bqk_ext[:, :])
            bv_s = res.tile([1, DG], F16)
            nc.sync.dma_start(out=bv_s, in_=bv_ext[:, :])
            mask_s = res.tile([128, 128], F16)
            nc.sync.dma_start(out=mask_s, in_=mask_ext[:, :])
            ones_s = res.tile([1, 128], F16)
            nc.vector.memset(ones_s, 1.0)
            # one-hot selector rows for the sum-collect matmuls:
            # sel[0, h, 32h] = 1
            sel_s = res.tile([DH + 1, H_PER, 128], F16)
            nc.vector.memset(sel_s[DH : DH + 1, :, :], 0.0)
            for h in range(H_PER):
                nc.vector.memset(
                    sel_s[DH : DH + 1, h, 32 * h : 32 * h + 1], 1.0
                )

            # ---- projections ----------------------------------------------
            QT_s = res.tile([128, NDT, SQ], F16)
            KT_s = res.tile([128, NDT, SKV], F16)
            # V with a fused ones column per head: [kv, head, 64+1]
            V_s = res.tile([128, NKT, H_PER, DH + 1], F16)
            nc.vector.memset(V_s[:, :, :, DH : DH + 1], 1.0)

            psp_cm = tc.tile_pool(name="psp", bufs=8, space="PSUM")
            psp = psp_cm.__enter__()
            for dt in range(1):
                for which, w_s, dst, bias_col in (
                    (0, wq_s, QT_s, dt),
                    (1, wk_s, KT_s, NDT + dt),
                ):
                    src_kt = qT_kt if which == 0 else kvT_kt
                    p_qs = []
                    for qb in range(NQB):
                        p_t = psp.tile(
                            [128, QB], F32, tag="proj", name=f"p{which}{dt}{qb}"
                        )
                        p_qs.append(p_t)
                    for kt in range(NCT):
                        for qb in range(NQB):
                            nc.tensor.matmul(
                                p_qs[qb],
                                w_s[:, kt, 128 * dt : 128 * dt + 128],
                                src_kt[kt][:, QB * qb : QB * qb + QB],
                                start=(kt == 0),
                                stop=(kt == NCT - 1),
                            )
                    for qb in range(NQB):
                        nc.vector.tensor_scalar_add(
                            dst[:, dt, QB * qb : QB * qb + QB],
                            p_qs[qb],
                            bqk_s[:, bias_col : bias_col + 1],
                        )

            for it in range(NKT):
                p_v = psp.tile([128, DG], F32, tag="proj")
                for kt in range(NCT):
                    nc.tensor.matmul(
                        p_v,
                        kvT_kt[kt][:, KT * it : KT * it + KT],
                        wv_s[:, kt, :],
                        start=(kt == 0),
                        stop=False,
                    )
                nc.tensor.matmul(p_v, ones_s, bv_s, start=False, stop=True)
                nc.vector.tensor_copy(
                    V_s[:, it, :, 0:DH],
                    p_v.rearrange("p (h d) -> p h d", h=H_PER),
                )

            psp_cm.__exit__(None, None, None)

            # ---- attention: head pairs (0,1) and (2,3) --------------------
            pss_cm = tc.tile_pool(name="pss", bufs=2, space="PSUM")
            pss = pss_cm.__enter__()
            pso_cm = tc.tile_pool(name="pso", bufs=2, space="PSUM")
            pso = pso_cm.__enter__()
            psy_cm = tc.tile_pool(name="psy", bufs=2, space="PSUM")
            psy = psy_cm.__enter__()

            OT_s = res.tile([128, NDT, SQ], F16)

            def emit_y(qb):
                for mt in range(D // 128):
                    y_q = psy.tile([128, QB], F32, tag="ypsum", name="y_q")
                    for kt2 in range(NDT):
                        nc.tensor.matmul(
                            y_q,
                            wo_s[:, kt2, 128 * mt : 128 * mt + 128],
                            OT_s[:, kt2, QB * qb : QB * qb + QB],
                            start=(kt2 == 0),
                            stop=(kt2 == NDT - 1),
                        )
                    y_sb = yp.tile([128, QB], F16, tag="ysb", name="y_sb")
                    nc.vector.tensor_copy(y_sb, y_q)
                    nc.sync.dma_start(
                        out=yT_ext[
                            128 * mt : 128 * mt + 128, QB * qb : QB * qb + QB
                        ],
                        in_=y_sb,
                    )

            # deferred dt=1 projections, interleaved into the td=0
            # attention stream to fill PE bubbles while ACT ramps up
            proj1_steps = []

            def _mk_proj1():
                dt = 1
                for which, w_s, dst, bias_col in (
                    (0, wq_s, QT_s, dt),
                    (1, wk_s, KT_s, NDT + dt),
                ):
                    src_kt = qT_kt if which == 0 else kvT_kt
                    p_box = {}

                    def alloc(which=which):
                        p_box["t"] = psy.tile(
                            [128, QB], F32, tag="ypsum", name=f"pp1{which}"
                        )

                    for qb2 in range(NQB):
                        proj1_steps.append(alloc)
                        for kt in range(NCT):

                            def mm(kt=kt, qb2=qb2, w_s=w_s, src_kt=src_kt):
                                nc.tensor.matmul(
                                    p_box["t"],
                                    w_s[:, kt, 128 : 128 + 128],
                                    src_kt[kt][:, QB * qb2 : QB * qb2 + QB],
                                    start=(kt == 0),
                                    stop=(kt == NCT - 1),
                                )

                            proj1_steps.append(mm)

                        def bias(qb2=qb2, dst=dst, bias_col=bias_col):
                            nc.vector.tensor_scalar_add(
                                dst[:, 1, QB * qb2 : QB * qb2 + QB],
                                p_box["t"],
                                bqk_s[:, bias_col : bias_col + 1],
                            )

                        proj1_steps.append(bias)

            _mk_proj1()
            p1i = [0]

            def drain_proj1(n):
                while n > 0 and p1i[0] < len(proj1_steps):
                    proj1_steps[p1i[0]]()
                    p1i[0] += 1
                    n -= 1

            osb_all = {}

            def attn_pair(qb, td, interleave):
                n_it = 4 * qb + 4
                o_a = pso.tile([DH + 1, QB], F32, tag="opsum", name="o_a")
                o_b = pso.tile([DH + 1, QB], F32, tag="opsum", name="o_b")
                prev = None
                for it in range(n_it):
                    c_start = max(QB * qb, KT * it)
                    width = QB * (qb + 1) - c_start
                    co = c_start - QB * qb
                    s_pair = pss.tile(
                        [128, 2, QB], F32, tag="spair", name="s_pair"
                    )
                    nc.tensor.matmul(
                        s_pair[:, 0, 0:width],
                        KT_s[0:DH, td, KT * it : KT * it + KT],
                        QT_s[0:DH, td, c_start : c_start + width],
                        start=True,
                        stop=True,
                    )
                    nc.tensor.matmul(
                        s_pair[:, 1, 0:width],
                        KT_s[DH:128, td, KT * it : KT * it + KT],
                        QT_s[DH:128, td, c_start : c_start + width],
                        start=True,
                        stop=True,
                    )
                    p_pair = pp.tile([128, 2, QB], F16, tag="ptile", name="p_pair")
                    nc.scalar.activation(
                        p_pair[:, :, 0:width],
                        s_pair[:, :, 0:width],
                        mybir.ActivationFunctionType.Exp,
                    )
                    if it >= 4 * qb:  # diagonal block: causal mask
                        nc.vector.tensor_mul(
                            p_pair[:, :, 0:128],
                            p_pair[:, :, 0:128],
                            mask_s.unsqueeze(1).broadcast_to([128, 2, 128]),
                        )
                    if interleave:
                        drain_proj1(2)
                    # software pipeline: PV for the previous k-tile, so the
                    # PE never waits on the current exp
                    if prev is not None:
                        pp_prev, co_p, w_p, it_p = prev
                        nc.tensor.matmul(
                            o_a[:, co_p : co_p + w_p],
                            V_s[:, it_p, 2 * td, :],
                            pp_prev[:, 0, 0:w_p],
                            start=(it_p == 0),
                            stop=False,
                        )
                        nc.tensor.matmul(
                            o_b[:, co_p : co_p + w_p],
                            V_s[:, it_p, 2 * td + 1, :],
                            pp_prev[:, 1, 0:w_p],
                            start=(it_p == 0),
                            stop=False,
                        )
                    prev = (p_pair, co, width, it)
                pp_prev, co_p, w_p, it_p = prev
                nc.tensor.matmul(
                    o_a[:, co_p : co_p + w_p],
                    V_s[:, it_p, 2 * td, :],
                    pp_prev[:, 0, 0:w_p],
                    start=(it_p == 0),
                    stop=True,
                )
                nc.tensor.matmul(
                    o_b[:, co_p : co_p + w_p],
                    V_s[:, it_p, 2 * td + 1, :],
                    pp_prev[:, 1, 0:w_p],
                    start=(it_p == 0),
                    stop=True,
                )
                for i_half, o_ps in ((0, o_a), (1, o_b)):
                    h = 2 * td + i_half
                    o_sb = op.tile(
                        [DH + 1, QB], F16, tag=f"osb{h % 2}", name="o_sb"
                    )
                    nc.vector.tensor_copy(o_sb, o_ps)
                    osb_all[(qb, h)] = o_sb

            # phase A: head pair (0,1) with dt=1 projections interleaved
            for qb in range(NQB):
                attn_pair(qb, 0, True)
            drain_proj1(len(proj1_steps))

            # phase B: head pair (2,3); divisions and out-proj skewed one
            # q-block behind attention so they never stall the PE
            def emit_div(qb):
                c4 = psy.tile([128, QB], F32, tag="ypsum", name="c4")
                for h in range(H_PER):
                    nc.tensor.matmul(
                        c4,
                        sel_s[DH : DH + 1, h, :],
                        osb_all[(qb, h)][DH : DH + 1, :],
                        start=(h == 0),
                        stop=(h == H_PER - 1),
                    )
                r4 = rp.tile([128, QB], F32, tag="r4")
                nc.vector.reciprocal(r4, c4)
                for h in range(H_PER):
                    r_h = rp.tile([1, QB], F32, tag="rh")
                    nc.vector.tensor_copy(r_h, r4[32 * h : 32 * h + 1, :])
                    rb = rp.tile([DH, QB], F32, tag="rb")
                    nc.gpsimd.partition_broadcast(rb, r_h)
                    p0 = DH * (h % 2)
                    td2 = h // 2
                    nc.vector.tensor_mul(
                        OT_s[p0 : p0 + DH, td2, QB * qb : QB * qb + QB],
                        osb_all[(qb, h)][0:DH, :],
                        rb,
                    )

            for qb in range(NQB):
                attn_pair(qb, 1, False)
                emit_div(qb)
                if qb > 0:
                    emit_y(qb - 1)
            emit_y(NQB - 1)
            psy_cm.__exit__(None, None, None)
            pso_cm.__exit__(None, None, None)
            pss_cm.__exit__(None, None, None)

    nc.finalize()
    return nc


def _get_nc():
    if "nc" not in _CACHE:
        _CACHE["nc"] = _build()
    return _CACHE["nc"]


def _prep_core_inputs(c, query, key_value, Wq, bq, Wk, bk, Wv, bv, Wo, bo):
    b = c // HG
    hg = c % HG
    hs = slice(DG * hg, DG * hg + DG)
    scale = 1.0 / np.sqrt(DH)

    bqk = np.zeros((128, 2 * NDT), np.float32)
    bq_s = (bq[hs] * scale).astype(np.float32)
    bk_s = bk[hs].astype(np.float32)
    for dt in range(NDT):
        bqk[:, dt] = bq_s[128 * dt : 128 * dt + 128]
        bqk[:, NDT + dt] = bk_s[128 * dt : 128 * dt + 128]

    kk, qq = np.meshgrid(np.arange(128), np.arange(128), indexing="ij")
    mask = (qq >= kk).astype(np.float16)

    return {
        "qT": np.ascontiguousarray(query[b].T).astype(np.float16),
        "kvT": np.ascontiguousarray(key_value[b].T).astype(np.float16),
        "wqT": np.ascontiguousarray((Wq[hs, :] * scale).T).astype(np.float16),
        "wkT": np.ascontiguousarray(Wk[hs, :].T).astype(np.float16),
        "wvT": np.ascontiguousarray(Wv[hs, :].T).astype(np.float16),
        "woT": np.ascontiguousarray(Wo[:, hs].T).astype(np.float16),
        "bqk": bqk,
        "bv": bv[hs].reshape(1, DG).astype(np.float16),
        "mask": mask,
    }


def kernel(
    query,
    key_value,
    Wq,
    bq,
    Wk,
    bk,
    Wv,
    bv,
    Wo,
    bo,
    _trace=False,
):
    query = np.asarray(query)
    key_value = np.asarray(key_value)
    args = [np.asarray(a) for a in (Wq, bq, Wk, bk, Wv, bv, Wo, bo)]

    nc = _get_nc()
    in_maps = [
        _prep_core_inputs(c, query, key_value, *args) for c in range(N_CORES)
    ]
    res = run_bass_kernel_spmd(
        nc, in_maps, list(range(N_CORES)), trace=_trace
    )

    out = np.zeros((B, SQ, D), np.float32)
    for c in range(N_CORES):
        out[c // HG] += res.results[c]["yT"].astype(np.float32).T
    out += args[7].astype(np.float32)  # bo
    if _trace:
        return out, res
    return out



# revision 10
# speedup vs baseline: 1.0660x; 1.0660x over previous
"""Causal cross-attention Trainium2 kernel (v3).

Sharding: 8 cores = 2 (batch) x 4 (head-groups of 4 heads, tensor parallel).
Each core computes its batch's attention for its 4 heads plus the partial
(row-parallel) output projection; the host sums head-group partials + bo.

Device layout is fully transposed (Q^T/K^T = [d, seq], scores = S^T[k, q],
output Y^T = [D, SQ]) so the softmax denominator comes free as a fused
ones-column in the PV matmul. No max-subtraction in softmax: scores are
~N(0,1) (scale 1/sqrt(64) folded into Wq), raw exp is safe in fp32.

v3 structure (the thing that matters on TRN2 is keeping the PE streaming
continuously so its clock stays at the max p-state):
  - qT/kvT ship in q-block-major slabs [128, 4, 8, 512]; DMAs are issued
    in first-use order round-robin over sync/scalar/gpsimd queues, so the
    first projection matmul starts as soon as ~1.5MB has landed.
  - Only the minimal prefix (K dt0/kvb0, Q dt0/qb0, V it0-3) runs before
    attention; every remaining projection unit (8-9 matmuls + drain) is
    interleaved between attention iterations of phase A, and the
    out-projection m-tiles between iterations of phase B. The PE queue
    therefore always holds ready work and never idles waiting on the
    exp->PV chain.
  - Attention processes head pairs: the two K=64 score matmuls row-pack
    into PE row groups 0-63/64-127, share one PSUM tile, one strided ACT
    exp covers both; PV is software-pipelined one k-tile behind.
  - Denominators: span-64 selector matmuls collect each head's ones-row
    into base-0 PSUM tiles; reciprocal_approx_fast (~5x cheaper than
    reciprocal, ~18 correct bits) gives the divisors; causal mask
    multiplies run on gpsimd (SBUF-only op) to unload DVE.
"""

import sys

if "/opt/trn_rl_repo" not in sys.path:
    sys.path.insert(0, "/opt/trn_rl_repo")

import numpy as np

import concourse.mybir as mybir
import concourse.tile as tile
from concourse import bacc
from concourse.bass_utils import run_bass_kernel_spmd

# problem shapes (hardcoded)
B = 2
SQ = 2048
SKV = 2048
D = 1024
H = 16
DH = 64
N_CORES = 8
HG = 4  # head groups
H_PER = H // HG  # 4 heads per core
DG = H_PER * DH  # 256 dims per core

F16 = mybir.dt.float16
F32 = mybir.dt.float32

QB = 512  # q block (free dim per matmul)
KT = 128  # kv tile (partition dim)
NQB = SQ // QB  # 4
NKT = SKV // KT  # 16
NCT = D // 128  # 8 contraction tiles for projections
NDT = DG // 128  # 2 partition tiles for the 256 head dims

_CACHE = {}


def _build():
    nc = bacc.Bacc("TRN2", target_bir_lowering=False, debug=False)

    qT_ext = nc.dram_tensor("qT", [128, NQB, NCT, QB], F16, kind="ExternalInput")
    kvT_ext = nc.dram_tensor("kvT", [128, NQB, NCT, QB], F16, kind="ExternalInput")
    wqT_ext = nc.dram_tensor("wqT", [D, DG], F16, kind="ExternalInput")
    wkT_ext = nc.dram_tensor("wkT", [D, DG], F16, kind="ExternalInput")
    wvT_ext = nc.dram_tensor("wvT", [D, DG], F16, kind="ExternalInput")
    woT_ext = nc.dram_tensor("woT", [DG, D], F16, kind="ExternalInput")
    bqk_ext = nc.dram_tensor("bqk", [128, 2 * NDT], F32, kind="ExternalInput")
    bv_ext = nc.dram_tensor("bv", [1, DG], F16, kind="ExternalInput")
    mask_ext = nc.dram_tensor("mask", [128, 128], F16, kind="ExternalInput")
    yT_ext = nc.dram_tensor("yT", [D, SQ], F16, kind="ExternalOutput")

    with tile.TileContext(nc) as tc:
        with (
            tc.tile_pool(name="res", bufs=1) as res,
            tc.tile_pool(name="pp", bufs=6) as pp,
            tc.tile_pool(name="yp", bufs=6) as yp,
            tc.tile_pool(name="rp", bufs=4) as rp,
            tc.tile_pool(name="op", bufs=5) as op,
        ):
            # ---- input DMAs: first-use order, explicit queue placement ----
            # so the K-projection's gate (wk + kv slab 0) lands first and
            # each queue streams ~independently
            kvT_s = res.tile([128, NQB, NCT, QB], F16)
            qT_s = res.tile([128, NQB, NCT, QB], F16)
            wk_s = res.tile([128, NCT, DG], F16)
            wv_s = res.tile([128, NCT, DG], F16)
            wq_s = res.tile([128, NCT, DG], F16)
            wo_s = res.tile([128, NDT, D], F16)
            bqk_s = res.tile([128, 2 * NDT], F32)
            mask_s = res.tile([128, 128], F16)
            bv_s = res.tile([1, DG], F16)

            nc.sync.dma_start(out=kvT_s[:, 0], in_=kvT_ext[:, 0])
            nc.scalar.dma_start(
                out=wk_s, in_=wkT_ext.rearrange("(kt p) d -> p kt d", p=128)
            )
            nc.gpsimd.dma_start(out=bqk_s, in_=bqk_ext[:, :])
            nc.gpsimd.dma_start(out=mask_s, in_=mask_ext[:, :])
            nc.gpsimd.dma_start(out=bv_s, in_=bv_ext[:, :])
            nc.scalar.dma_start(
                out=wv_s, in_=wvT_ext.rearrange("(kt p) d -> p kt d", p=128)
            )
            nc.sync.dma_start(
                out=wq_s, in_=wqT_ext.rearrange("(kt p) d -> p kt d", p=128)
            )
            nc.scalar.dma_start(out=qT_s[:, 0], in_=qT_ext[:, 0])
            nc.gpsimd.dma_start(out=kvT_s[:, 2], in_=kvT_ext[:, 2])
            nc.sync.dma_start(out=kvT_s[:, 1], in_=kvT_ext[:, 1])
            nc.scalar.dma_start(out=qT_s[:, 2], in_=qT_ext[:, 2])
            nc.sync.dma_start(out=qT_s[:, 1], in_=qT_ext[:, 1])
            nc.gpsimd.dma_start(out=qT_s[:, 3], in_=qT_ext[:, 3])
            nc.sync.dma_start(out=kvT_s[:, 3], in_=kvT_ext[:, 3])
            nc.scalar.dma_start(
                out=wo_s, in_=woT_ext.rearrange("(kt p) m -> p kt m", p=128)
            )

            ones_s = res.tile([1, 128], F16)
            nc.vector.memset(ones_s, 1.0)
            # span-64 selector (row DH to match osb base partition): collects
            # a denominator row into output partitions 0-63
            sel_s = res.tile([DH + 1, DH], F16)
            nc.vector.memset(sel_s[DH : DH + 1, :], 1.0)

            QT_s = res.tile([128, NDT, SQ], F16)
            KT_s = res.tile([128, NDT, SKV], F16)
            V_s = res.tile([128, NKT, H_PER, DH + 1], F16)
            nc.vector.memset(V_s[:, :, :, DH : DH + 1], 1.0)
            OT_s = res.tile([128, NDT, SQ], F16)

            # ---- projection units (each: 8-9 matmuls + PSUM drain) --------
            psy_cm = tc.tile_pool(name="psy", bufs=2, space="PSUM")
            psy = psy_cm.__enter__()

            def unit_qk(which, dt, sb):
                # one [128, 512] projection tile: Q or K, dims dt, block sb
                w_s = wq_s if which == 0 else wk_s
                src = qT_s if which == 0 else kvT_s
                dst = QT_s if which == 0 else KT_s
                p_t = psy.tile([128, QB], F32, tag="ypsum", name=f"p{which}{dt}{sb}")
                for kt in range(NCT):
                    nc.tensor.matmul(
                        p_t,
                        w_s[:, kt, 128 * dt : 128 * dt + 128],
                        src[:, sb, kt, :],
                        start=(kt == 0),
                        stop=(kt == NCT - 1),
                    )
                nc.vector.tensor_scalar_add(
                    dst[:, dt, QB * sb : QB * sb + QB],
                    p_t,
                    bqk_s[:, (0 if which == 0 else NDT) + dt : (0 if which == 0 else NDT) + dt + 1],
                )

            def unit_v(it):
                pv = psy.tile([128, DG], F32, tag="ypsum", name=f"pv{it}")
                for kt in range(NCT):
                    nc.tensor.matmul(
                        pv,
                        kvT_s[:, it // 4, kt, 128 * (it % 4) : 128 * (it % 4) + 128],
                        wv_s[:, kt, :],
                        start=(kt == 0),
                        stop=False,
                    )
                nc.tensor.matmul(pv, ones_s, bv_s, start=False, stop=True)
                nc.vector.tensor_copy(
                    V_s[:, it, :, 0:DH],
                    pv.rearrange("p (h d) -> p h d", h=H_PER),
                )

            # ---- attention ------------------------------------------------
            pss_cm = tc.tile_pool(name="pss", bufs=2, space="PSUM")
            pss = pss_cm.__enter__()
            pso_cm = tc.tile_pool(name="pso", bufs=2, space="PSUM")
            pso = pso_cm.__enter__()

            osb_all = {}

            def attn_pair(qb, td, drain):
                n_it = 4 * qb + 4
                o_a = pso.tile([DH + 1, QB], F32, tag="opsum", name="o_a")
                o_b = pso.tile([DH + 1, QB], F32, tag="opsum", name="o_b")
                prev = None
                for it in range(n_it):
                    c_start = max(QB * qb, KT * it)
                    width = QB * (qb + 1) - c_start
                    co = c_start - QB * qb
                    s_pair = pss.tile(
                        [128, 2, QB], F32, tag="spair", name="s_pair"
                    )
                    nc.tensor.matmul(
                        s_pair[:, 0, 0:width],
                        KT_s[0:DH, td, KT * it : KT * it + KT],
                        QT_s[0:DH, td, c_start : c_start + width],
                        start=True,
                        stop=True,
                    )
                    nc.tensor.matmul(
                        s_pair[:, 1, 0:width],
                        KT_s[DH:128, td, KT * it : KT * it + KT],
                        QT_s[DH:128, td, c_start : c_start + width],
                        start=True,
                        stop=True,
                    )
                    p_pair = pp.tile([128, 2, QB], F16, tag="ptile", name="p_pair")
                    nc.scalar.activation(
                        p_pair[:, :, 0:width],
                        s_pair[:, :, 0:width],
                        mybir.ActivationFunctionType.Exp,
                    )
                    if it >= 4 * qb:  # diagonal block: causal mask (on gpsimd)
                        nc.gpsimd.tensor_mul(
                            p_pair[:, :, 0:128],
                            p_pair[:, :, 0:128],
                            mask_s.unsqueeze(1).broadcast_to([128, 2, 128]),
                        )
                    if drain:
                        drain.pop(0)()
                    # software pipeline: PV for the previous k-tile, so the
                    # PE never waits on the current exp
                    if prev is not None:
                        pp_prev, co_p, w_p, it_p = prev
                        nc.tensor.matmul(
                            o_a[:, co_p : co_p + w_p],
                            V_s[:, it_p, 2 * td, :],
                            pp_prev[:, 0, 0:w_p],
                            start=(it_p == 0),
                            stop=False,
                        )
                        nc.tensor.matmul(
                            o_b[:, co_p : co_p + w_p],
                            V_s[:, it_p, 2 * td + 1, :],
                            pp_prev[:, 1, 0:w_p],
                            start=(it_p == 0),
                            stop=False,
                        )
                    prev = (p_pair, co, width, it)
                pp_prev, co_p, w_p, it_p = prev
                nc.tensor.matmul(
                    o_a[:, co_p : co_p + w_p],
                    V_s[:, it_p, 2 * td, :],
                    pp_prev[:, 0, 0:w_p],
                    start=(it_p == 0),
                    stop=True,
                )
                nc.tensor.matmul(
                    o_b[:, co_p : co_p + w_p],
                    V_s[:, it_p, 2 * td + 1, :],
                    pp_prev[:, 1, 0:w_p],
                    start=(it_p == 0),
                    stop=True,
                )
                for i_half, o_ps in ((0, o_a), (1, o_b)):
                    h = 2 * td + i_half
                    o_sb = op.tile(
                        [DH + 1, QB], F16, tag=f"osb{h % 2}", name="o_sb"
                    )
                    nc.vector.tensor_copy(o_sb, o_ps)
                    osb_all[(qb, h)] = o_sb

            def emit_div(qb, td):
                # per head: span-collect the ones-row into partitions 0-63 of
                # its own PSUM tile, approx-reciprocal, divide O into OT
                for i_half in range(2):
                    h = 2 * td + i_half
                    osb = osb_all[(qb, h)]
                    cd = psy.tile([DH, QB], F32, tag="ypsum", name="cd")
                    nc.tensor.matmul(
                        cd,
                        sel_s[DH : DH + 1, :],
                        osb[DH : DH + 1, :],
                        start=True,
                        stop=True,
                    )
                    rr = rp.tile([DH, QB], F32, tag="rr")
                    nc.vector.reciprocal_approx_fast(out=rr, in_=cd)
                    nc.vector.tensor_mul(
                        OT_s[
                            DH * i_half : DH * i_half + DH,
                            td,
                            QB * qb : QB * qb + QB,
                        ],
                        osb[0:DH, :],
                        rr,
                    )

            def unit_y(qb, mt):
                y_q = psy.tile([128, QB], F32, tag="ypsum", name="y_q")
                for kt2 in range(NDT):
                    nc.tensor.matmul(
                        y_q,
                        wo_s[:, kt2, 128 * mt : 128 * mt + 128],
                        OT_s[:, kt2, QB * qb : QB * qb + QB],
                        start=(kt2 == 0),
                        stop=(kt2 == NDT - 1),
                    )
                y_sb = yp.tile([128, QB], F16, tag="ysb", name="y_sb")
                nc.vector.tensor_copy(y_sb, y_q)
                eng = nc.sync if mt % 2 == 0 else nc.gpsimd
                eng.dma_start(
                    out=yT_ext[
                        128 * mt : 128 * mt + 128, QB * qb : QB * qb + QB
                    ],
                    in_=y_sb,
                )

            def nop():
                pass

            # ---- schedule -------------------------------------------------
            # prefix: just enough projection for attention (qb0, td0)
            unit_qk(1, 0, 0)  # K dt0 kvb0
            for it in range(4):
                unit_v(it)
            unit_qk(0, 0, 0)  # Q dt0 qb0

            # phase A (td0): every remaining projection unit interleaved
            # between attention iterations. Hard ordering: Q(dt0,qb) before
            # the pair (first score reads it); K(dt0,qb) and V(4qb..4qb+3)
            # early in the pair's drain (consumed from iteration 4qb on).
            for qb in range(NQB):
                if qb > 0:
                    unit_qk(0, 0, qb)
                drain = []
                if qb > 0:
                    drain.append(lambda qb=qb: unit_qk(1, 0, qb))
                    for it in range(4 * qb, 4 * qb + 4):
                        drain.append(lambda it=it: unit_v(it))
                drain.append(lambda qb=qb: unit_qk(1, 1, qb))
                drain.append(lambda qb=qb: unit_qk(0, 1, qb))
                attn_pair(qb, 0, drain)
                while drain:
                    drain.pop(0)()
                if qb > 0:
                    emit_div(qb - 1, 0)
            emit_div(NQB - 1, 0)

            # phase B (td1): divisions + out-projection skewed one q-block
            for qb in range(NQB):
                drain = []
                if qb > 0:
                    drain.append(lambda qb=qb: emit_div(qb - 1, 1))
                    drain.append(nop)
                    for mt in range(D // 128):
                        drain.append(lambda qb=qb, mt=mt: unit_y(qb - 1, mt))
                attn_pair(qb, 1, drain)
                while drain:
                    drain.pop(0)()
            emit_div(NQB - 1, 1)
            for mt in range(D // 128):
                unit_y(NQB - 1, mt)

            pso_cm.__exit__(None, None, None)
            pss_cm.__exit__(None, None, None)
            psy_cm.__exit__(None, None, None)

    nc.finalize()
    return nc


def _get_nc():
    if "nc" not in _CACHE:
        _CACHE["nc"] = _build()
    return _CACHE["nc"]


def _slab(x):
    """fp32/fp16 [D, S] -> slab-major [128, NQB, NCT, QB] fp16."""
    return np.ascontiguousarray(
        x.reshape(NCT, 128, NQB, QB).transpose(1, 2, 0, 3)
    ).astype(np.float16)


def _prep_core_inputs(c, query, key_value, Wq, bq, Wk, bk, Wv, bv, Wo, bo):
    b = c // HG
    hg = c % HG
    hs = slice(DG * hg, DG * hg + DG)
    scale = 1.0 / np.sqrt(DH)

    bqk = np.zeros((128, 2 * NDT), np.float32)
    bq_s = (bq[hs] * scale).astype(np.float32)
    bk_s = bk[hs].astype(np.float32)
    for dt in range(NDT):
        bqk[:, dt] = bq_s[128 * dt : 128 * dt + 128]
        bqk[:, NDT + dt] = bk_s[128 * dt : 128 * dt + 128]

    kk, qq = np.meshgrid(np.arange(128), np.arange(128), indexing="ij")
    mask = (qq >= kk).astype(np.float16)

    return {
        "qT": _slab(query[b].T),
        "kvT": _slab(key_value[b].T),
        "wqT": np.ascontiguousarray((Wq[hs, :] * scale).T).astype(np.float16),
        "wkT": np.ascontiguousarray(Wk[hs, :].T).astype(np.float16),
        "wvT": np.ascontiguousarray(Wv[hs, :].T).astype(np.float16),
        "woT": np.ascontiguousarray(Wo[:, hs].T).astype(np.float16),
        "bqk": bqk,
        "bv": bv[hs].reshape(1, DG).astype(np.float16),
        "mask": mask,
    }


def kernel(
    query,
    key_value,
    Wq,
    bq,
    Wk,
    bk,
    Wv,
    bv,
    Wo,
    bo,
    _trace=False,
):
    query = np.asarray(query)
    key_value = np.asarray(key_value)
    args = [np.asarray(a) for a in (Wq, bq, Wk, bk, Wv, bv, Wo, bo)]

    nc = _get_nc()
    in_maps = [
        _prep_core_inputs(c, query, key_value, *args) for c in range(N_CORES)
    ]
    res = run_bass_kernel_spmd(
        nc, in_maps, list(range(N_CORES)), trace=_trace
    )

    out = np.zeros((B, SQ, D), np.float32)
    for c in range(N_CORES):
        out[c // HG] += res.results[c]["yT"].astype(np.float32).T
    out += args[7].astype(np.float32)  # bo
    if _trace:
        return out, res
    return out


# revision 13
# speedup vs baseline: 1.1955x; 1.1215x over previous
"""Causal cross-attention Trainium2 kernel (v3).

Sharding: 8 cores = 2 (batch) x 4 (head-groups of 4 heads, tensor parallel).
Each core computes its batch's attention for its 4 heads plus the partial
(row-parallel) output projection; the host sums head-group partials + bo.

Device layout is fully transposed (Q^T/K^T = [d, seq], scores = S^T[k, q],
output Y^T = [D, SQ]) so the softmax denominator comes free as a fused
ones-column in the PV matmul. No max-subtraction in softmax: scores are
~N(0,1) (scale 1/sqrt(64) folded into Wq), raw exp is safe in fp32.

v3 structure (the thing that matters on TRN2 is keeping the PE streaming
continuously so its clock stays at the max p-state):
  - qT/kvT ship in q-block-major slabs [128, 4, 8, 512]; DMAs are issued
    in first-use order round-robin over sync/scalar/gpsimd queues, so the
    first projection matmul starts as soon as ~1.5MB has landed.
  - Only the minimal prefix (K dt0/kvb0, Q dt0/qb0, V it0-3) runs before
    attention; every remaining projection unit (8-9 matmuls + drain) is
    interleaved between attention iterations of phase A, and the
    out-projection m-tiles between iterations of phase B. The PE queue
    therefore always holds ready work and never idles waiting on the
    exp->PV chain.
  - Attention processes head pairs: the two K=64 score matmuls row-pack
    into PE row groups 0-63/64-127, share one PSUM tile, one strided ACT
    exp covers both; PV is software-pipelined one k-tile behind.
  - Denominators: span-64 selector matmuls collect each head's ones-row
    into base-0 PSUM tiles; reciprocal_approx_fast (~5x cheaper than
    reciprocal, ~18 correct bits) gives the divisors; causal mask
    multiplies run on gpsimd (SBUF-only op) to unload DVE.
"""

import sys

if "/opt/trn_rl_repo" not in sys.path:
    sys.path.insert(0, "/opt/trn_rl_repo")

import numpy as np

import concourse.mybir as mybir
import concourse.tile as tile
from concourse import bacc
from concourse.bass_utils import run_bass_kernel_spmd

# problem shapes (hardcoded)
B = 2
SQ = 2048
SKV = 2048
D = 1024
H = 16
DH = 64
N_CORES = 8
HG = 4  # head groups
H_PER = H // HG  # 4 heads per core
DG = H_PER * DH  # 256 dims per core

F16 = mybir.dt.float16
F32 = mybir.dt.float32

QB = 512  # q block (free dim per matmul)
KT = 128  # kv tile (partition dim)
NQB = SQ // QB  # 4
NKT = SKV // KT  # 16
NCT = D // 128  # 8 contraction tiles for projections
NDT = DG // 128  # 2 partition tiles for the 256 head dims

_CACHE = {}


def _build():
    nc = bacc.Bacc("TRN2", target_bir_lowering=False, debug=False)

    qT_ext = nc.dram_tensor("qT", [128, NQB, NCT, QB], F16, kind="ExternalInput")
    kvT_ext = nc.dram_tensor("kvT", [128, NQB, NCT, QB], F16, kind="ExternalInput")
    wqT_ext = nc.dram_tensor("wqT", [D, DG], F16, kind="ExternalInput")
    wkT_ext = nc.dram_tensor("wkT", [D, DG], F16, kind="ExternalInput")
    wvT_ext = nc.dram_tensor("wvT", [D, DG], F16, kind="ExternalInput")
    woT_ext = nc.dram_tensor("woT", [DG, D], F16, kind="ExternalInput")
    bqk_ext = nc.dram_tensor("bqk", [128, 2 * NDT], F32, kind="ExternalInput")
    bv_ext = nc.dram_tensor("bv", [1, DG], F16, kind="ExternalInput")
    mask_ext = nc.dram_tensor("mask", [128, 128], F16, kind="ExternalInput")
    yT_ext = nc.dram_tensor("yT", [D, SQ], F16, kind="ExternalOutput")

    with tile.TileContext(nc) as tc:
        with (
            tc.tile_pool(name="res", bufs=1) as res,
            tc.tile_pool(name="pp", bufs=6) as pp,
            tc.tile_pool(name="yp", bufs=6) as yp,
            tc.tile_pool(name="rp", bufs=4) as rp,
            tc.tile_pool(name="op", bufs=5) as op,
        ):
            # ---- input DMAs: first-use order, explicit queue placement ----
            # so the K-projection's gate (wk + kv slab 0) lands first and
            # each queue streams ~independently
            kvT_s = res.tile([128, NQB, NCT, QB], F16)
            qT_s = res.tile([128, NQB, NCT, QB], F16)
            wk_s = res.tile([128, NCT, DG], F16)
            wv_s = res.tile([128, NCT, DG], F16)
            wq_s = res.tile([128, NCT, DG], F16)
            wo_s = res.tile([128, NDT, D], F16)
            bqk_s = res.tile([128, 2 * NDT], F32)
            mask_s = res.tile([128, 128], F16)
            bv_s = res.tile([1, DG], F16)

            # gpsimd carries the causal-mask multiplies during attention, so
            # it gets NO dma work; scalar only the early weight loads (exp
            # owns that queue later); everything else streams on sync.
            nc.sync.dma_start(out=kvT_s[:, 0], in_=kvT_ext[:, 0])
            nc.scalar.dma_start(
                out=wk_s, in_=wkT_ext.rearrange("(kt p) d -> p kt d", p=128)
            )
            nc.sync.dma_start(out=bqk_s, in_=bqk_ext[:, :])
            nc.scalar.dma_start(
                out=wv_s, in_=wvT_ext.rearrange("(kt p) d -> p kt d", p=128)
            )
            nc.sync.dma_start(out=mask_s, in_=mask_ext[:, :])
            nc.sync.dma_start(out=bv_s, in_=bv_ext[:, :])
            nc.sync.dma_start(
                out=wq_s, in_=wqT_ext.rearrange("(kt p) d -> p kt d", p=128)
            )
            nc.scalar.dma_start(out=qT_s[:, 0], in_=qT_ext[:, 0])
            nc.sync.dma_start(out=kvT_s[:, 1], in_=kvT_ext[:, 1])
            nc.scalar.dma_start(out=qT_s[:, 1], in_=qT_ext[:, 1])
            nc.sync.dma_start(out=kvT_s[:, 2], in_=kvT_ext[:, 2])
            nc.scalar.dma_start(out=qT_s[:, 2], in_=qT_ext[:, 2])
            nc.sync.dma_start(out=kvT_s[:, 3], in_=kvT_ext[:, 3])
            nc.scalar.dma_start(out=qT_s[:, 3], in_=qT_ext[:, 3])
            nc.sync.dma_start(
                out=wo_s, in_=woT_ext.rearrange("(kt p) m -> p kt m", p=128)
            )

            ones_s = res.tile([1, 128], F16)
            nc.vector.memset(ones_s, 1.0)
            # span-64 selector (row DH to match osb base partition): collects
            # a denominator row into output partitions 0-63
            sel_s = res.tile([DH + 1, DH], F16)
            nc.vector.memset(sel_s[DH : DH + 1, :], 1.0)

            QT_s = res.tile([128, NDT, SQ], F16)
            KT_s = res.tile([128, NDT, SKV], F16)
            V_s = res.tile([128, NKT, H_PER, DH + 1], F16)
            nc.vector.memset(V_s[:, :, :, DH : DH + 1], 1.0)
            OT_s = res.tile([128, NDT, SQ], F16)

            # ---- projection units (each: 8-9 matmuls + PSUM drain) --------
            psy_cm = tc.tile_pool(name="psy", bufs=2, space="PSUM")
            psy = psy_cm.__enter__()

            def unit_qk(which, dt, sb):
                # one [128, 512] projection tile: Q or K, dims dt, block sb
                w_s = wq_s if which == 0 else wk_s
                src = qT_s if which == 0 else kvT_s
                dst = QT_s if which == 0 else KT_s
                p_t = psy.tile([128, QB], F32, tag="ypsum", name=f"p{which}{dt}{sb}")
                for kt in range(NCT):
                    nc.tensor.matmul(
                        p_t,
                        w_s[:, kt, 128 * dt : 128 * dt + 128],
                        src[:, sb, kt, :],
                        start=(kt == 0),
                        stop=(kt == NCT - 1),
                    )
                nc.vector.tensor_scalar_add(
                    dst[:, dt, QB * sb : QB * sb + QB],
                    p_t,
                    bqk_s[:, (0 if which == 0 else NDT) + dt : (0 if which == 0 else NDT) + dt + 1],
                )

            def unit_v(it):
                pv = psy.tile([128, DG], F32, tag="ypsum", name=f"pv{it}")
                for kt in range(NCT):
                    nc.tensor.matmul(
                        pv,
                        kvT_s[:, it // 4, kt, 128 * (it % 4) : 128 * (it % 4) + 128],
                        wv_s[:, kt, :],
                        start=(kt == 0),
                        stop=False,
                    )
                nc.tensor.matmul(pv, ones_s, bv_s, start=False, stop=True)
                nc.vector.tensor_copy(
                    V_s[:, it, :, 0:DH],
                    pv.rearrange("p (h d) -> p h d", h=H_PER),
                )

            # ---- attention ------------------------------------------------
            pss_cm = tc.tile_pool(name="pss", bufs=2, space="PSUM")
            pss = pss_cm.__enter__()
            pso_cm = tc.tile_pool(name="pso", bufs=2, space="PSUM")
            pso = pso_cm.__enter__()

            osb_all = {}

            def attn_pair(qb, td, drain):
                n_it = 4 * qb + 4
                o_a = pso.tile([DH + 1, QB], F32, tag="opsum", name="o_a")
                o_b = pso.tile([DH + 1, QB], F32, tag="opsum", name="o_b")
                prev = None
                for it in range(n_it):
                    c_start = max(QB * qb, KT * it)
                    width = QB * (qb + 1) - c_start
                    co = c_start - QB * qb
                    s_pair = pss.tile(
                        [128, 2, QB], F32, tag="spair", name="s_pair"
                    )
                    nc.tensor.matmul(
                        s_pair[:, 0, 0:width],
                        KT_s[0:DH, td, KT * it : KT * it + KT],
                        QT_s[0:DH, td, c_start : c_start + width],
                        start=True,
                        stop=True,
                    )
                    nc.tensor.matmul(
                        s_pair[:, 1, 0:width],
                        KT_s[DH:128, td, KT * it : KT * it + KT],
                        QT_s[DH:128, td, c_start : c_start + width],
                        start=True,
                        stop=True,
                    )
                    p_pair = pp.tile([128, 2, QB], F16, tag="ptile", name="p_pair")
                    nc.scalar.activation(
                        p_pair[:, :, 0:width],
                        s_pair[:, :, 0:width],
                        mybir.ActivationFunctionType.Exp,
                    )
                    if it >= 4 * qb:  # diagonal block: causal mask (on gpsimd)
                        nc.gpsimd.tensor_mul(
                            p_pair[:, :, 0:128],
                            p_pair[:, :, 0:128],
                            mask_s.unsqueeze(1).broadcast_to([128, 2, 128]),
                        )
                    if drain:
                        drain.pop(0)()
                    # software pipeline: PV for the previous k-tile, so the
                    # PE never waits on the current exp
                    if prev is not None:
                        pp_prev, co_p, w_p, it_p = prev
                        nc.tensor.matmul(
                            o_a[:, co_p : co_p + w_p],
                            V_s[:, it_p, 2 * td, :],
                            pp_prev[:, 0, 0:w_p],
                            start=(it_p == 0),
                            stop=False,
                        )
                        nc.tensor.matmul(
                            o_b[:, co_p : co_p + w_p],
                            V_s[:, it_p, 2 * td + 1, :],
                            pp_prev[:, 1, 0:w_p],
                            start=(it_p == 0),
                            stop=False,
                        )
                    prev = (p_pair, co, width, it)
                pp_prev, co_p, w_p, it_p = prev
                nc.tensor.matmul(
                    o_a[:, co_p : co_p + w_p],
                    V_s[:, it_p, 2 * td, :],
                    pp_prev[:, 0, 0:w_p],
                    start=(it_p == 0),
                    stop=True,
                )
                nc.tensor.matmul(
                    o_b[:, co_p : co_p + w_p],
                    V_s[:, it_p, 2 * td + 1, :],
                    pp_prev[:, 1, 0:w_p],
                    start=(it_p == 0),
                    stop=True,
                )
                for i_half, o_ps in ((0, o_a), (1, o_b)):
                    h = 2 * td + i_half
                    o_sb = op.tile(
                        [DH + 1, QB], F16, tag=f"osb{h % 2}", name="o_sb"
                    )
                    nc.vector.tensor_copy(o_sb, o_ps)
                    osb_all[(qb, h)] = o_sb

            def emit_div(qb, td):
                # per head: span-collect the ones-row into partitions 0-63 of
                # its own PSUM tile, approx-reciprocal, divide O into OT
                for i_half in range(2):
                    h = 2 * td + i_half
                    osb = osb_all[(qb, h)]
                    cd = psy.tile([DH, QB], F32, tag="ypsum", name="cd")
                    nc.tensor.matmul(
                        cd,
                        sel_s[DH : DH + 1, :],
                        osb[DH : DH + 1, :],
                        start=True,
                        stop=True,
                    )
                    rr = rp.tile([DH, QB], F32, tag="rr")
                    nc.vector.reciprocal_approx_fast(out=rr, in_=cd)
                    nc.vector.tensor_mul(
                        OT_s[
                            DH * i_half : DH * i_half + DH,
                            td,
                            QB * qb : QB * qb + QB,
                        ],
                        osb[0:DH, :],
                        rr,
                    )

            def unit_y(qb, mt, pool=None):
                y_q = (pool or psy).tile(
                    [128, QB], F32, tag="ypsum" if pool is None else "opsum",
                    name="y_q",
                )
                for kt2 in range(NDT):
                    nc.tensor.matmul(
                        y_q,
                        wo_s[:, kt2, 128 * mt : 128 * mt + 128],
                        OT_s[:, kt2, QB * qb : QB * qb + QB],
                        start=(kt2 == 0),
                        stop=(kt2 == NDT - 1),
                    )
                y_sb = yp.tile([128, QB], F16, tag="ysb", name="y_sb")
                nc.vector.tensor_copy(y_sb, y_q)
                nc.sync.dma_start(
                    out=yT_ext[
                        128 * mt : 128 * mt + 128, QB * qb : QB * qb + QB
                    ],
                    in_=y_sb,
                )

            def nop():
                pass

            # ---- schedule -------------------------------------------------
            # prefix: just enough projection for attention (qb0, td0)
            unit_qk(1, 0, 0)  # K dt0 kvb0
            for it in range(4):
                unit_v(it)
            unit_qk(0, 0, 0)  # Q dt0 qb0

            # phase A (td0): every remaining projection unit interleaved
            # between attention iterations. Hard ordering: Q(dt0,qb) before
            # the pair (first score reads it); K(dt0,qb) and V(4qb..4qb+3)
            # early in the pair's drain (consumed from iteration 4qb on).
            for qb in range(NQB):
                if qb > 0:
                    unit_qk(0, 0, qb)
                drain = []
                if qb > 0:
                    drain.append(lambda qb=qb: unit_qk(1, 0, qb))
                    for it in range(4 * qb, 4 * qb + 4):
                        drain.append(lambda it=it: unit_v(it))
                drain.append(lambda qb=qb: unit_qk(1, 1, qb))
                drain.append(lambda qb=qb: unit_qk(0, 1, qb))
                attn_pair(qb, 0, drain)
                while drain:
                    drain.pop(0)()
                if qb > 0:
                    emit_div(qb - 1, 0)
            emit_div(NQB - 1, 0)

            # phase B (td1): divisions + out-projection skewed one q-block
            for qb in range(NQB):
                drain = []
                if qb > 0:
                    drain.append(lambda qb=qb: emit_div(qb - 1, 1))
                    drain.append(nop)
                    for mt in range(D // 128):
                        drain.append(lambda qb=qb, mt=mt: unit_y(qb - 1, mt))
                attn_pair(qb, 1, drain)
                while drain:
                    drain.pop(0)()
            # tail: the last q-block's out-projection borrows the attention
            # accumulators' freed PSUM banks for a 4-deep pipeline
            emit_div(NQB - 1, 1)
            for mt in range(D // 128):
                unit_y(NQB - 1, mt, pool=(pso if mt % 2 else None))

            pso_cm.__exit__(None, None, None)
            pss_cm.__exit__(None, None, None)
            psy_cm.__exit__(None, None, None)

    nc.finalize()
    return nc


def _get_nc():
    if "nc" not in _CACHE:
        _CACHE["nc"] = _build()
    return _CACHE["nc"]


def _slab(x):
    """fp32/fp16 [D, S] -> slab-major [128, NQB, NCT, QB] fp16."""
    return np.ascontiguousarray(
        x.reshape(NCT, 128, NQB, QB).transpose(1, 2, 0, 3)
    ).astype(np.float16)


def _prep_core_inputs(c, query, key_value, Wq, bq, Wk, bk, Wv, bv, Wo, bo):
    b = c // HG
    hg = c % HG
    hs = slice(DG * hg, DG * hg + DG)
    scale = 1.0 / np.sqrt(DH)

    bqk = np.zeros((128, 2 * NDT), np.float32)
    bq_s = (bq[hs] * scale).astype(np.float32)
    bk_s = bk[hs].astype(np.float32)
    for dt in range(NDT):
        bqk[:, dt] = bq_s[128 * dt : 128 * dt + 128]
        bqk[:, NDT + dt] = bk_s[128 * dt : 128 * dt + 128]

    kk, qq = np.meshgrid(np.arange(128), np.arange(128), indexing="ij")
    mask = (qq >= kk).astype(np.float16)

    return {
        "qT": _slab(query[b].T),
        "kvT": _slab(key_value[b].T),
        "wqT": np.ascontiguousarray((Wq[hs, :] * scale).T).astype(np.float16),
        "wkT": np.ascontiguousarray(Wk[hs, :].T).astype(np.float16),
        "wvT": np.ascontiguousarray(Wv[hs, :].T).astype(np.float16),
        "woT": np.ascontiguousarray(Wo[:, hs].T).astype(np.float16),
        "bqk": bqk,
        "bv": bv[hs].reshape(1, DG).astype(np.float16),
        "mask": mask,
    }


def kernel(
    query,
    key_value,
    Wq,
    bq,
    Wk,
    bk,
    Wv,
    bv,
    Wo,
    bo,
    _trace=False,
):
    query = np.asarray(query)
    key_value = np.asarray(key_value)
    args = [np.asarray(a) for a in (Wq, bq, Wk, bk, Wv, bv, Wo, bo)]

    nc = _get_nc()
    in_maps = [
        _prep_core_inputs(c, query, key_value, *args) for c in range(N_CORES)
    ]
    res = run_bass_kernel_spmd(
        nc, in_maps, list(range(N_CORES)), trace=_trace
    )

    out = np.zeros((B, SQ, D), np.float32)
    for c in range(N_CORES):
        out[c // HG] += res.results[c]["yT"].astype(np.float32).T
    out += args[7].astype(np.float32)  # bo
    if _trace:
        return out, res
    return out
